# revision 18
# baseline (speedup 1.0000x reference)
"""Trainium2 Bass kernel for the attention-gated MLP (nn_AFS_12549894439384).

Data-parallel over batch across 8 NeuronCores; all parameters replicated.
All matmuls run in bf16 with fp32 PSUM accumulation.

Per-core compute (local batch BL=1024) in transposed [feature, batch] layout:
    eT   = tanh(We.T @ xT + be)                  [32, BL]
    z_a  = Wa[:,:,a].T-tile @ eT  (K=32)         [128, BL] per F-tile
    att  = sigmoid(tanh(z1+ba1) - tanh(z0+ba0))  [128, BL]
    wT   = xT * att                              [128, BL]
    hT   = relu(W1-tile.T @ wT + b1)             [128, BL] per H-tile (K=2048)
    out  = hT-slice.T @ W2 + b2                  [128, 1000] per B-tile (K=2048)

The gate loop is paced by the Scalar engine's tanh/tanh/sigmoid chain
(~3.1us per F-tile); the first two H-tiles of the W1 matmul are interleaved
into it with a 2-iteration lag so the TensorEngine stays busy during it.
"""

import numpy as np
import ml_dtypes

import concourse.bass as bass
import concourse.mybir as mybir
import concourse.tile as tile
from concourse import bacc
from concourse.bass_utils import run_bass_kernel_spmd

BF16 = ml_dtypes.bfloat16

F_DIM, E_NODE, A_NODE = 2048, 32, 2
HIDDEN, CLASSES, BATCH = 2048, 1000, 8192
NCORES = 8
BL = BATCH // NCORES          # 1024 local batch rows per core
P = 128
FT = F_DIM // P               # 16 feature tiles
HT = HIDDEN // P              # 16 hidden tiles
BT = BL // P                  # 8 local batch tiles
CH = CLASSES // 2             # 500
NG0 = 2                       # H-tiles interleaved into the gate loop
LAG = 3                       # gate-loop iterations between wT and its B-matmuls

_CACHED = {}


def build_nc():
    dt = mybir.dt
    nc = bacc.Bacc("TRN2", target_bir_lowering=False, debug=False)

    xT_d = nc.declare_dram_parameter("xT", [FT, P, BL], dt.bfloat16, isOutput=False)
    wet_d = nc.declare_dram_parameter("wet", [P, FT * P], dt.bfloat16, isOutput=False)
    be_d = nc.declare_dram_parameter("be", [P, 1], dt.float32, isOutput=False)
    waq_d = nc.declare_dram_parameter("waq", [P, F_DIM], dt.bfloat16, isOutput=False)
    w1_d = nc.declare_dram_parameter("w1t", [HT, P, F_DIM], dt.bfloat16, isOutput=False)
    b1_d = nc.declare_dram_parameter("b1t", [P, HT], dt.float32, isOutput=False)
    w2_d = nc.declare_dram_parameter("w2r", [HT, P, CLASSES], dt.bfloat16, isOutput=False)
    b2_d = nc.declare_dram_parameter("b2r", [P, CLASSES], dt.float32, isOutput=False)

    out_d = nc.declare_dram_parameter("out", [BT, P, CLASSES], dt.float32, isOutput=True)
    att_d = nc.declare_dram_parameter("attT", [FT, P, BL], dt.bfloat16, isOutput=True)

    AFT = mybir.ActivationFunctionType
    HB = BL // 2              # 512, matmul N limit

    with tile.TileContext(nc) as tc:
        from contextlib import ExitStack

        with ExitStack() as ctx:
            const_pool = ctx.enter_context(tc.tile_pool(name="const", bufs=1))
            xt_pool = ctx.enter_context(tc.tile_pool(name="xt", bufs=FT))
            wt_pool = ctx.enter_context(tc.tile_pool(name="wt", bufs=FT))
            ht_pool = ctx.enter_context(tc.tile_pool(name="ht", bufs=HT))
            w1_pool = ctx.enter_context(tc.tile_pool(name="w1", bufs=4))
            w2_pool = ctx.enter_context(tc.tile_pool(name="w2", bufs=HT))
            act_pool = ctx.enter_context(tc.tile_pool(name="acts", bufs=2))
            out_pool = ctx.enter_context(tc.tile_pool(name="outs", bufs=3))

            # ---- load xT tiles first (they gate the first matmuls); split
            # the first tiles into half-DMAs and alternate issue between the
            # SP and Activation HWDGE sequencers so transfers ramp at once.
            xt = []
            for f in range(FT):
                xt_t = xt_pool.tile([P, BL], dt.bfloat16, tag="xt", name=f"xt{f}")
                eng = nc.sync if f % 2 == 0 else nc.scalar
                if f < 4:
                    eng.dma_start(xt_t[:, 0:BL // 2], xT_d[f, :, 0:BL // 2])
                    eng2 = nc.scalar if f % 2 == 0 else nc.sync
                    eng2.dma_start(xt_t[:, BL // 2:BL], xT_d[f, :, BL // 2:BL])
                else:
                    eng.dma_start(xt_t[:], xT_d[f, :, :])
                xt.append(xt_t)
                if f == 0:
                    wet_sb = const_pool.tile([P, FT * P], dt.bfloat16)
                    nc.sync.dma_start(wet_sb[:], wet_d[:, :])

            # ---- remaining constants / small weights ----
            be_sb = const_pool.tile([P, 1], dt.float32)
            nc.sync.dma_start(be_sb[:], be_d[:, :])
            waq_sb = const_pool.tile([P, F_DIM], dt.bfloat16)
            nc.sync.dma_start(waq_sb[:], waq_d[:, :])
            b1_sb = const_pool.tile([P, HT], dt.float32)
            nc.sync.dma_start(b1_sb[:], b1_d[:, :])
            b2_sb = const_pool.tile([P, CLASSES], dt.float32)
            nc.sync.dma_start(b2_sb[:], b2_d[:, :])

            # W1 tiles for the interleaved H-group, prefetched early
            w1_g0 = []
            for h in range(NG0):
                w1_sb = w1_pool.tile([P, F_DIM], dt.bfloat16, tag="w1", name=f"w1g0_{h}")
                nc.sync.dma_start(w1_sb[:], w1_d[h, :, :])
                w1_g0.append(w1_sb)

            # ---- eT = tanh(We.T @ xT + be), replicated at partitions
            # 0-31 and 64-95 so the two gate matmuls can run as concurrent
            # 32-row PE tiles (T0 / T8) ----
            e_sb = const_pool.tile([P, BL], dt.bfloat16)
            with tc.tile_pool(name="psE", bufs=1, space="PSUM") as psE:
                pe = psE.tile([P, BL], dt.float32, tag="pe", name="pe")
                for f in range(FT):
                    lhs = wet_sb[:, f * P:(f + 1) * P]
                    nc.tensor.matmul(pe[:, 0:HB], lhs, xt[f][:, 0:HB],
                                     start=(f == 0), stop=(f == FT - 1))
                    nc.tensor.matmul(pe[:, HB:BL], lhs, xt[f][:, HB:BL],
                                     start=(f == 0), stop=(f == FT - 1))
                nc.scalar.activation(e_sb[:], pe[:], AFT.Tanh, bias=be_sb[:, 0:1])

            # ---- gate loop (Scalar-chain paced) + interleaved B-matmuls ----
            wt_tiles = [None] * FT
            ht_g0 = []
            with (
                tc.tile_pool(name="psZ", bufs=1, space="PSUM") as psZ,
                tc.tile_pool(name="psB0", bufs=1, space="PSUM") as psB0,
            ):
                phg = []
                for h in range(NG0):
                    phg.append(psB0.tile([P, BL], dt.float32, tag=f"phg{h}",
                                         name=f"phg{h}"))

                for fi in range(FT + LAG):
                    if fi < FT:
                        f = fi
                        K1 = E_NODE + 1
                        zp = psZ.tile([P, 2 * BL], dt.float32, tag="zp", name=f"zp_{f}")
                        l0 = waq_sb[0:K1, f * P:(f + 1) * P]
                        l1 = waq_sb[64:64 + K1, f * P:(f + 1) * P]
                        nc.tensor.matmul(zp[:, 0:HB], l0, e_sb[0:K1, 0:HB],
                                         start=True, stop=True, tile_position=(0, 0))
                        nc.tensor.matmul(zp[:, BL:BL + HB], l1, e_sb[64:64 + K1, 0:HB],
                                         start=True, stop=True, tile_position=(64, 0))
                        nc.tensor.matmul(zp[:, HB:BL], l0, e_sb[0:K1, HB:BL],
                                         start=True, stop=True, tile_position=(0, 0))
                        nc.tensor.matmul(zp[:, BL + HB:2 * BL], l1, e_sb[64:64 + K1, HB:BL],
                                         start=True, stop=True, tile_position=(64, 0))
                        t01 = act_pool.tile([P, 2 * BL], dt.float16, tag="t01", name=f"t01_{f}")
                        nc.scalar.activation(t01[:], zp[:], AFT.Tanh)
                        if f % 2 == 0:
                            dp = act_pool.tile([P, 2 * BL], dt.float16, tag="dp", name=f"dp_{f}")
                            d_sl = dp[:, 0:BL]
                        else:
                            d_sl = dp[:, BL:2 * BL]
                        nc.vector.tensor_sub(d_sl[:], t01[:, BL:2 * BL], t01[:, 0:BL])
                        if f % 2 == 1:
                            attp = act_pool.tile([P, 2 * BL], dt.bfloat16, tag="attp", bufs=2,
                                                 name=f"attp_{f}")
                            nc.scalar.activation(attp[:], dp[:], AFT.Sigmoid)
                            for g, fg in ((0, f - 1), (1, f)):
                                att_sl = attp[:, g * BL:(g + 1) * BL]
                                wt_t = wt_pool.tile([P, BL], dt.bfloat16, tag="wt",
                                                    name=f"wt{fg}")
                                nc.vector.tensor_mul(wt_t[:], xt[fg][:], att_sl[:])
                                nc.sync.dma_start(att_d[fg, :, :], att_sl[:])
                                wt_tiles[fg] = wt_t

                    if fi >= LAG:
                        f2 = fi - LAG
                        for h in range(NG0):
                            lhs = w1_g0[h][:, f2 * P:(f2 + 1) * P]
                            nc.tensor.matmul(phg[h][:, 0:HB], lhs,
                                             wt_tiles[f2][:, 0:HB],
                                             start=(f2 == 0), stop=(f2 == FT - 1))
                            nc.tensor.matmul(phg[h][:, HB:BL], lhs,
                                             wt_tiles[f2][:, HB:BL],
                                             start=(f2 == 0), stop=(f2 == FT - 1))

                for h in range(NG0):
                    ht_t = ht_pool.tile([P, BL], dt.bfloat16, tag="ht", name=f"htg{h}")
                    nc.scalar.activation(ht_t[:, 0:HB], phg[h][:, 0:HB], AFT.Relu,
                                         bias=b1_sb[:, h:h + 1])
                    nc.scalar.activation(ht_t[:, HB:BL], phg[h][:, HB:BL], AFT.Relu,
                                         bias=b1_sb[:, h:h + 1])
                    ht_g0.append(ht_t)

            # ---- prefetch W2 ----
            w2_sb = []
            for h in range(HT):
                w2_t = w2_pool.tile([P, CLASSES], dt.bfloat16, tag="w2", name=f"w2_{h}")
                nc.sync.dma_start(w2_t[:], w2_d[h, :, :])
                w2_sb.append(w2_t)

            # ---- phase B remainder: hT = relu(W1.T @ wT + b1) for h >= NG0 ----
            ht_tiles = list(ht_g0)
            with tc.tile_pool(name="psB", bufs=2, space="PSUM") as psB:
                for h in range(NG0, HT):
                    w1_sb = w1_pool.tile([P, F_DIM], dt.bfloat16, tag="w1",
                                         name=f"w1_{h}")
                    nc.sync.dma_start(w1_sb[:], w1_d[h, :, :])
                    ph0 = psB.tile([P, HB], dt.float32, tag="ph0", name=f"ph0_{h}")
                    ph1 = psB.tile([P, HB], dt.float32, tag="ph1", name=f"ph1_{h}")
                    for f in range(FT):
                        lhs = w1_sb[:, f * P:(f + 1) * P]
                        nc.tensor.matmul(ph0[:], lhs, wt_tiles[f][:, 0:HB],
                                         start=(f == 0), stop=(f == FT - 1))
                        nc.tensor.matmul(ph1[:], lhs, wt_tiles[f][:, HB:BL],
                                         start=(f == 0), stop=(f == FT - 1))
                    ht_t = ht_pool.tile([P, BL], dt.bfloat16, tag="ht", name=f"ht{h}")
                    nc.scalar.activation(ht_t[:, 0:HB], ph0[:], AFT.Relu,
                                         bias=b1_sb[:, h:h + 1])
                    nc.scalar.activation(ht_t[:, HB:BL], ph1[:], AFT.Relu,
                                         bias=b1_sb[:, h:h + 1])
                    ht_tiles.append(ht_t)

            # ---- phase C: out = hT.T @ W2 + b2 ----
            with tc.tile_pool(name="psC", bufs=2, space="PSUM") as psC:
                for b in range(BT):
                    po0 = psC.tile([P, CH], dt.float32, tag="o0", name=f"po0_{b}")
                    po1 = psC.tile([P, CH], dt.float32, tag="o1", name=f"po1_{b}")
                    for h in range(HT):
                        lhs = ht_tiles[h][:, b * P:(b + 1) * P]
                        nc.tensor.matmul(po0[:], lhs, w2_sb[h][:, 0:CH],
                                         start=(h == 0), stop=(h == HT - 1))
                        nc.tensor.matmul(po1[:], lhs, w2_sb[h][:, CH:CLASSES],
                                         start=(h == 0), stop=(h == HT - 1))
                    o_sb = out_pool.tile([P, CLASSES], dt.float32, tag="o", name=f"o{b}")
                    nc.vector.tensor_add(o_sb[:, 0:CH], po0[:], b2_sb[:, 0:CH])
                    nc.vector.tensor_add(o_sb[:, CH:CLASSES], po1[:],
                                         b2_sb[:, CH:CLASSES])
                    nc.sync.dma_start(out_d[b, :, :], o_sb[:])

    nc.compile()
    return nc


def _prep_inputs(x, We, be, Wa, ba, W1, b1, W2, b2):
    f32 = np.float32

    xt_all = np.ascontiguousarray(
        x.astype(BF16).reshape(NCORES, BL, FT, P).transpose(0, 2, 3, 1)
    )
    wet3 = np.zeros((P, FT, P), dtype=np.float32)
    wer = We.reshape(FT, P, E_NODE).transpose(1, 0, 2)   # [P, FT, E]
    wet3[:, :, 0:E_NODE] = wer
    wet3[:, :, 64:64 + E_NODE] = wer
    wet = np.ascontiguousarray(wet3.reshape(P, FT * P)).astype(BF16)
    be_c = np.zeros((P, 1), dtype=f32)
    be_c[0:E_NODE, 0] = be
    be_c[64:64 + E_NODE, 0] = be
    be_c[E_NODE, 0] = 20.0      # tanh(20) == 1.0: constant-one row for bias
    be_c[64 + E_NODE, 0] = 20.0
    waq = np.zeros((P, F_DIM), dtype=np.float32)
    waq[0:E_NODE] = Wa[:, :, 0].T
    waq[E_NODE] = ba[:, 0]
    waq[64:64 + E_NODE] = Wa[:, :, 1].T
    waq[64 + E_NODE] = ba[:, 1]
    waq = waq.astype(BF16)
    w1t = np.ascontiguousarray(
        W1.astype(BF16).reshape(FT, P, HT, P).transpose(2, 1, 0, 3).reshape(HT, P, F_DIM)
    )
    b1t = np.ascontiguousarray(b1.reshape(HT, P).T).astype(f32)
    w2r = np.ascontiguousarray(W2.reshape(HT, P, CLASSES)).astype(BF16)
    b2r = np.ascontiguousarray(np.broadcast_to(b2.astype(f32), (P, CLASSES)))

    shared = {
        "wet": wet, "be": be_c, "waq": waq,
        "w1t": w1t, "b1t": b1t,
        "w2r": w2r, "b2r": b2r,
    }
    return [dict(shared, xT=np.ascontiguousarray(xt_all[c])) for c in range(NCORES)]


def run_on_hw(inputs, trace=False):
    if "nc" not in _CACHED:
        _CACHED["nc"] = build_nc()
    nc = _CACHED["nc"]
    in_maps = _prep_inputs(**inputs)
    res = run_bass_kernel_spmd(nc, in_maps, core_ids=list(range(NCORES)), trace=trace)

    outs = []
    atts = []
    for c in range(NCORES):
        outs.append(res.results[c]["out"].reshape(BL, CLASSES))
        att_c = res.results[c]["attT"]  # [FT, P, BL] bf16
        atts.append(att_c.transpose(2, 0, 1).reshape(BL, F_DIM))
    output = np.concatenate(outs, axis=0).astype(np.float32)
    attention = np.concatenate(atts, axis=0).astype(np.float32)
    return (output, attention), res


def kernel(**inputs):
    (output, attention), _ = run_on_hw(inputs, trace=False)
    return (output, attention)


# revision 19
# speedup vs baseline: 1.2055x; 1.2055x over previous
"""Trainium2 Bass kernel for the attention-gated MLP (nn_AFS_12549894439384).

Data-parallel over batch across 8 NeuronCores; all parameters replicated.
All matmuls run in bf16 with fp32 PSUM accumulation.

Per-core compute (local batch BL=1024) in transposed [feature, batch] layout:
    eT   = tanh(We.T @ xT + be)                  [32, BL]
    z_a  = Wa[:,:,a].T-tile @ eT  (K=32)         [128, BL] per F-tile
    att  = sigmoid(tanh(z1+ba1) - tanh(z0+ba0))  [128, BL]
    wT   = xT * att                              [128, BL]
    hT   = relu(W1-tile.T @ wT + b1)             [128, BL] per H-tile (K=2048)
    out  = hT-slice.T @ W2 + b2                  [128, 1000] per B-tile (K=2048)

The gate loop is paced by the Scalar engine's tanh/tanh/sigmoid chain
(~3.1us per F-tile); the first two H-tiles of the W1 matmul are interleaved
into it with a 2-iteration lag so the TensorEngine stays busy during it.
"""

import numpy as np
import ml_dtypes

import concourse.bass as bass
import concourse.mybir as mybir
import concourse.tile as tile
from concourse import bacc
from concourse.bass_utils import run_bass_kernel_spmd

BF16 = ml_dtypes.bfloat16

F_DIM, E_NODE, A_NODE = 2048, 32, 2
HIDDEN, CLASSES, BATCH = 2048, 1000, 8192
NCORES = 8
BL = BATCH // NCORES          # 1024 local batch rows per core
P = 128
FT = F_DIM // P               # 16 feature tiles
HT = HIDDEN // P              # 16 hidden tiles
BT = BL // P                  # 8 local batch tiles
CH = CLASSES // 2             # 500
NG0 = 2                       # H-tiles interleaved into the gate loop
LAG = 2                       # gate-loop iterations between wT and its B-matmuls

_CACHED = {}


def build_nc():
    dt = mybir.dt
    nc = bacc.Bacc("TRN2", target_bir_lowering=False, debug=False)

    xT_d = nc.declare_dram_parameter("xT", [FT, P, BL], dt.bfloat16, isOutput=False)
    wet_d = nc.declare_dram_parameter("wet", [P, FT * 96], dt.bfloat16, isOutput=False)
    be_d = nc.declare_dram_parameter("be", [96, 1], dt.float32, isOutput=False)
    waq_d = nc.declare_dram_parameter("waq", [96, F_DIM], dt.bfloat16, isOutput=False)
    ba0_d = nc.declare_dram_parameter("ba0t", [P, FT], dt.float32, isOutput=False)
    ba1_d = nc.declare_dram_parameter("ba1t", [P, FT], dt.float32, isOutput=False)
    w1_d = nc.declare_dram_parameter("w1t", [HT, P, F_DIM], dt.bfloat16, isOutput=False)
    b1_d = nc.declare_dram_parameter("b1t", [P, HT], dt.float32, isOutput=False)
    w2_d = nc.declare_dram_parameter("w2r", [HT, P, CLASSES], dt.bfloat16, isOutput=False)
    b2_d = nc.declare_dram_parameter("b2r", [P, CLASSES], dt.float32, isOutput=False)

    out_d = nc.declare_dram_parameter("out", [BT, P, CLASSES], dt.float32, isOutput=True)
    att_d = nc.declare_dram_parameter("attT", [FT, P, BL], dt.bfloat16, isOutput=True)

    AFT = mybir.ActivationFunctionType
    HB = BL // 2              # 512, matmul N limit

    with tile.TileContext(nc) as tc:
        from contextlib import ExitStack

        with ExitStack() as ctx:
            const_pool = ctx.enter_context(tc.tile_pool(name="const", bufs=1))
            xt_pool = ctx.enter_context(tc.tile_pool(name="xt", bufs=FT))
            wt_pool = ctx.enter_context(tc.tile_pool(name="wt", bufs=FT))
            ht_pool = ctx.enter_context(tc.tile_pool(name="ht", bufs=HT))
            w1_pool = ctx.enter_context(tc.tile_pool(name="w1", bufs=4))
            w2_pool = ctx.enter_context(tc.tile_pool(name="w2", bufs=HT))
            act_pool = ctx.enter_context(tc.tile_pool(name="acts", bufs=2))
            out_pool = ctx.enter_context(tc.tile_pool(name="outs", bufs=3))

            # ---- load xT tiles first (they gate the first matmuls); split
            # the first tiles into half-DMAs and alternate issue between the
            # SP and Activation HWDGE sequencers so transfers ramp at once.
            xt = []
            for f in range(FT):
                xt_t = xt_pool.tile([P, BL], dt.bfloat16, tag="xt", name=f"xt{f}")
                eng = nc.sync if f % 2 == 0 else nc.scalar
                if f < 4:
                    eng.dma_start(xt_t[:, 0:BL // 2], xT_d[f, :, 0:BL // 2])
                    eng2 = nc.scalar if f % 2 == 0 else nc.sync
                    eng2.dma_start(xt_t[:, BL // 2:BL], xT_d[f, :, BL // 2:BL])
                else:
                    eng.dma_start(xt_t[:], xT_d[f, :, :])
                xt.append(xt_t)
                if f == 0:
                    wet_sb = const_pool.tile([P, FT * 96], dt.bfloat16)
                    nc.sync.dma_start(wet_sb[:], wet_d[:, :])

            # ---- remaining constants / small weights ----
            be_sb = const_pool.tile([96, 1], dt.float32)
            nc.sync.dma_start(be_sb[:], be_d[:, :])
            waq_sb = const_pool.tile([96, F_DIM], dt.bfloat16)
            nc.sync.dma_start(waq_sb[:], waq_d[:, :])
            ba0_sb = const_pool.tile([P, FT], dt.float32)
            nc.sync.dma_start(ba0_sb[:], ba0_d[:, :])
            ba1_sb = const_pool.tile([P, FT], dt.float32)
            nc.sync.dma_start(ba1_sb[:], ba1_d[:, :])
            b1_sb = const_pool.tile([P, HT], dt.float32)
            nc.sync.dma_start(b1_sb[:], b1_d[:, :])
            b2_sb = const_pool.tile([P, CLASSES], dt.float32)
            nc.sync.dma_start(b2_sb[:], b2_d[:, :])

            # W1 tiles for the interleaved H-group, prefetched early
            w1_g0 = []
            for h in range(NG0):
                w1_sb = w1_pool.tile([P, F_DIM], dt.bfloat16, tag="w1", name=f"w1g0_{h}")
                nc.sync.dma_start(w1_sb[:], w1_d[h, :, :])
                w1_g0.append(w1_sb)

            # ---- eT = tanh(We.T @ xT + be), replicated at partitions
            # 0-31 and 64-95 so the two gate matmuls can run as concurrent
            # 32-row PE tiles (T0 / T8) ----
            e_sb = const_pool.tile([96, BL], dt.bfloat16)
            with tc.tile_pool(name="psE", bufs=1, space="PSUM") as psE:
                pe = psE.tile([96, BL], dt.float32, tag="pe", name="pe")
                for f in range(FT):
                    lhs = wet_sb[:, f * 96:(f + 1) * 96]
                    nc.tensor.matmul(pe[:, 0:HB], lhs, xt[f][:, 0:HB],
                                     start=(f == 0), stop=(f == FT - 1))
                    nc.tensor.matmul(pe[:, HB:BL], lhs, xt[f][:, HB:BL],
                                     start=(f == 0), stop=(f == FT - 1))
                nc.scalar.activation(e_sb[:], pe[:], AFT.Tanh, bias=be_sb[:, 0:1])

            # ---- gate loop (Scalar-chain paced) + interleaved B-matmuls ----
            wt_tiles = [None] * FT
            ht_g0 = []
            with (
                tc.tile_pool(name="psZ", bufs=1, space="PSUM") as psZ,
                tc.tile_pool(name="psB0", bufs=1, space="PSUM") as psB0,
            ):
                phg = []
                for h in range(NG0):
                    phg.append(psB0.tile([P, BL], dt.float32, tag=f"phg{h}",
                                         name=f"phg{h}"))

                for fi in range(FT + LAG):
                    if fi < FT:
                        f = fi
                        zp0 = psZ.tile([P, BL], dt.float32, tag="zp0", name=f"zp0_{f}")
                        zp1 = psZ.tile([P, BL], dt.float32, tag="zp1", name=f"zp1_{f}")
                        l0 = waq_sb[0:E_NODE, f * P:(f + 1) * P]
                        l1 = waq_sb[64:64 + E_NODE, f * P:(f + 1) * P]
                        nc.tensor.matmul(zp0[:, 0:HB], l0, e_sb[0:E_NODE, 0:HB],
                                         start=True, stop=True, tile_position=(0, 0))
                        nc.tensor.matmul(zp1[:, 0:HB], l1, e_sb[64:64 + E_NODE, 0:HB],
                                         start=True, stop=True, tile_position=(64, 0))
                        nc.tensor.matmul(zp0[:, HB:BL], l0, e_sb[0:E_NODE, HB:BL],
                                         start=True, stop=True, tile_position=(0, 0))
                        nc.tensor.matmul(zp1[:, HB:BL], l1, e_sb[64:64 + E_NODE, HB:BL],
                                         start=True, stop=True, tile_position=(64, 0))
                        t0 = act_pool.tile([P, BL], dt.float16, tag="t0", name=f"t0_{f}")
                        t1 = act_pool.tile([P, BL], dt.float16, tag="t1", name=f"t1_{f}")
                        nc.scalar.activation(t0[:], zp0[:], AFT.Tanh, bias=ba0_sb[:, f:f + 1])
                        nc.scalar.activation(t1[:], zp1[:], AFT.Tanh, bias=ba1_sb[:, f:f + 1])
                        d_t = act_pool.tile([P, BL], dt.float16, tag="d", name=f"d_{f}")
                        nc.vector.tensor_sub(d_t[:], t1[:], t0[:])
                        att_sb = act_pool.tile([P, BL], dt.bfloat16, tag="att", bufs=3,
                                               name=f"att_{f}")
                        nc.scalar.activation(att_sb[:], d_t[:], AFT.Sigmoid)
                        wt_t = wt_pool.tile([P, BL], dt.bfloat16, tag="wt", name=f"wt{f}")
                        nc.vector.tensor_mul(wt_t[:], xt[f][:], att_sb[:])
                        nc.sync.dma_start(att_d[f, :, :], att_sb[:])
                        wt_tiles[f] = wt_t

                    if fi >= LAG:
                        f2 = fi - LAG
                        for h in range(NG0):
                            lhs = w1_g0[h][:, f2 * P:(f2 + 1) * P]
                            nc.tensor.matmul(phg[h][:, 0:HB], lhs,
                                             wt_tiles[f2][:, 0:HB],
                                             start=(f2 == 0), stop=(f2 == FT - 1))
                            nc.tensor.matmul(phg[h][:, HB:BL], lhs,
                                             wt_tiles[f2][:, HB:BL],
                                             start=(f2 == 0), stop=(f2 == FT - 1))

                for h in range(NG0):
                    ht_t = ht_pool.tile([P, BL], dt.bfloat16, tag="ht", name=f"htg{h}")
                    nc.scalar.activation(ht_t[:, 0:HB], phg[h][:, 0:HB], AFT.Relu,
                                         bias=b1_sb[:, h:h + 1])
                    nc.scalar.activation(ht_t[:, HB:BL], phg[h][:, HB:BL], AFT.Relu,
                                         bias=b1_sb[:, h:h + 1])
                    ht_g0.append(ht_t)

            # ---- prefetch W2 ----
            w2_sb = []
            for h in range(HT):
                w2_t = w2_pool.tile([P, CLASSES], dt.bfloat16, tag="w2", name=f"w2_{h}")
                nc.sync.dma_start(w2_t[:], w2_d[h, :, :])
                w2_sb.append(w2_t)

            # ---- phase B remainder: hT = relu(W1.T @ wT + b1) for h >= NG0 ----
            ht_tiles = list(ht_g0)
            with tc.tile_pool(name="psB", bufs=2, space="PSUM") as psB:
                for h in range(NG0, HT):
                    w1_sb = w1_pool.tile([P, F_DIM], dt.bfloat16, tag="w1",
                                         name=f"w1_{h}")
                    nc.sync.dma_start(w1_sb[:], w1_d[h, :, :])
                    ph0 = psB.tile([P, HB], dt.float32, tag="ph0", name=f"ph0_{h}")
                    ph1 = psB.tile([P, HB], dt.float32, tag="ph1", name=f"ph1_{h}")
                    for f in range(FT):
                        lhs = w1_sb[:, f * P:(f + 1) * P]
                        nc.tensor.matmul(ph0[:], lhs, wt_tiles[f][:, 0:HB],
                                         start=(f == 0), stop=(f == FT - 1))
                        nc.tensor.matmul(ph1[:], lhs, wt_tiles[f][:, HB:BL],
                                         start=(f == 0), stop=(f == FT - 1))
                    ht_t = ht_pool.tile([P, BL], dt.bfloat16, tag="ht", name=f"ht{h}")
                    nc.scalar.activation(ht_t[:, 0:HB], ph0[:], AFT.Relu,
                                         bias=b1_sb[:, h:h + 1])
                    nc.scalar.activation(ht_t[:, HB:BL], ph1[:], AFT.Relu,
                                         bias=b1_sb[:, h:h + 1])
                    ht_tiles.append(ht_t)

            # ---- phase C: out = hT.T @ W2 + b2 ----
            with tc.tile_pool(name="psC", bufs=2, space="PSUM") as psC:
                for b in range(BT):
                    po0 = psC.tile([P, CH], dt.float32, tag="o0", name=f"po0_{b}")
                    po1 = psC.tile([P, CH], dt.float32, tag="o1", name=f"po1_{b}")
                    for h in range(HT):
                        lhs = ht_tiles[h][:, b * P:(b + 1) * P]
                        nc.tensor.matmul(po0[:], lhs, w2_sb[h][:, 0:CH],
                                         start=(h == 0), stop=(h == HT - 1))
                        nc.tensor.matmul(po1[:], lhs, w2_sb[h][:, CH:CLASSES],
                                         start=(h == 0), stop=(h == HT - 1))
                    o_sb = out_pool.tile([P, CLASSES], dt.float32, tag="o", name=f"o{b}")
                    nc.vector.tensor_add(o_sb[:, 0:CH], po0[:], b2_sb[:, 0:CH])
                    nc.vector.tensor_add(o_sb[:, CH:CLASSES], po1[:],
                                         b2_sb[:, CH:CLASSES])
                    nc.sync.dma_start(out_d[b, :, :], o_sb[:])

    nc.compile()
    return nc


def _prep_inputs(x, We, be, Wa, ba, W1, b1, W2, b2):
    f32 = np.float32

    xt_all = np.ascontiguousarray(
        x.astype(BF16).reshape(NCORES, BL, FT, P).transpose(0, 2, 3, 1)
    )
    wet3 = np.zeros((P, FT, 96), dtype=np.float32)
    wer = We.reshape(FT, P, E_NODE).transpose(1, 0, 2)   # [P, FT, E]
    wet3[:, :, 0:E_NODE] = wer
    wet3[:, :, 64:64 + E_NODE] = wer
    wet = np.ascontiguousarray(wet3.reshape(P, FT * 96)).astype(BF16)
    be_c = np.zeros((96, 1), dtype=f32)
    be_c[0:E_NODE, 0] = be
    be_c[64:64 + E_NODE, 0] = be
    waq = np.zeros((96, F_DIM), dtype=np.float32)
    waq[0:E_NODE] = Wa[:, :, 0].T
    waq[64:64 + E_NODE] = Wa[:, :, 1].T
    waq = waq.astype(BF16)
    ba0t = np.ascontiguousarray(ba[:, 0].reshape(FT, P).T).astype(f32)
    ba1t = np.ascontiguousarray(ba[:, 1].reshape(FT, P).T).astype(f32)
    w1t = np.ascontiguousarray(
        W1.astype(BF16).reshape(FT, P, HT, P).transpose(2, 1, 0, 3).reshape(HT, P, F_DIM)
    )
    b1t = np.ascontiguousarray(b1.reshape(HT, P).T).astype(f32)
    w2r = np.ascontiguousarray(W2.reshape(HT, P, CLASSES)).astype(BF16)
    b2r = np.ascontiguousarray(np.broadcast_to(b2.astype(f32), (P, CLASSES)))

    shared = {
        "wet": wet, "be": be_c, "waq": waq,
        "ba0t": ba0t, "ba1t": ba1t, "w1t": w1t, "b1t": b1t,
        "w2r": w2r, "b2r": b2r,
    }
    return [dict(shared, xT=np.ascontiguousarray(xt_all[c])) for c in range(NCORES)]


def run_on_hw(inputs, trace=False):
    if "nc" not in _CACHED:
        _CACHED["nc"] = build_nc()
    nc = _CACHED["nc"]
    in_maps = _prep_inputs(**inputs)
    res = run_bass_kernel_spmd(nc, in_maps, core_ids=list(range(NCORES)), trace=trace)

    outs = []
    atts = []
    for c in range(NCORES):
        outs.append(res.results[c]["out"].reshape(BL, CLASSES))
        att_c = res.results[c]["attT"]  # [FT, P, BL] bf16
        atts.append(att_c.transpose(2, 0, 1).reshape(BL, F_DIM))
    output = np.concatenate(outs, axis=0).astype(np.float32)
    attention = np.concatenate(atts, axis=0).astype(np.float32)
    return (output, attention), res


def kernel(**inputs):
    (output, attention), _ = run_on_hw(inputs, trace=False)
    return (output, attention)


# revision 20
# speedup vs baseline: 1.2064x; 1.0008x over previous
"""Trainium2 Bass kernel for the attention-gated MLP (nn_AFS_12549894439384).

Data-parallel over batch across 8 NeuronCores; all parameters replicated.
All matmuls run in bf16 with fp32 PSUM accumulation.

Per-core compute (local batch BL=1024) in transposed [feature, batch] layout:
    eT   = tanh(We.T @ xT + be)                  [32, BL]
    z_a  = Wa[:,:,a].T-tile @ eT  (K=32)         [128, BL] per F-tile
    att  = sigmoid(tanh(z1+ba1) - tanh(z0+ba0))  [128, BL]
    wT   = xT * att                              [128, BL]
    hT   = relu(W1-tile.T @ wT + b1)             [128, BL] per H-tile (K=2048)
    out  = hT-slice.T @ W2 + b2                  [128, 1000] per B-tile (K=2048)

The gate loop is paced by the Scalar engine's tanh/tanh/sigmoid chain
(~3.1us per F-tile); the first two H-tiles of the W1 matmul are interleaved
into it with a 2-iteration lag so the TensorEngine stays busy during it.
"""

import numpy as np
import ml_dtypes

import concourse.bass as bass
import concourse.mybir as mybir
import concourse.tile as tile
from concourse import bacc
from concourse.bass_utils import run_bass_kernel_spmd

BF16 = ml_dtypes.bfloat16

F_DIM, E_NODE, A_NODE = 2048, 32, 2
HIDDEN, CLASSES, BATCH = 2048, 1000, 8192
NCORES = 8
BL = BATCH // NCORES          # 1024 local batch rows per core
P = 128
FT = F_DIM // P               # 16 feature tiles
HT = HIDDEN // P              # 16 hidden tiles
BT = BL // P                  # 8 local batch tiles
CH = CLASSES // 2             # 500
NG0 = 2                       # H-tiles interleaved into the gate loop
LAG = 3                       # gate-loop iterations between wT and its B-matmuls

_CACHED = {}


def build_nc():
    dt = mybir.dt
    nc = bacc.Bacc("TRN2", target_bir_lowering=False, debug=False)

    xT_d = nc.declare_dram_parameter("xT", [FT, P, BL], dt.bfloat16, isOutput=False)
    wet_d = nc.declare_dram_parameter("wet", [P, FT * 96], dt.bfloat16, isOutput=False)
    be_d = nc.declare_dram_parameter("be", [96, 1], dt.float32, isOutput=False)
    waq_d = nc.declare_dram_parameter("waq", [96, F_DIM], dt.bfloat16, isOutput=False)
    ba0_d = nc.declare_dram_parameter("ba0t", [P, FT], dt.float32, isOutput=False)
    ba1_d = nc.declare_dram_parameter("ba1t", [P, FT], dt.float32, isOutput=False)
    w1_d = nc.declare_dram_parameter("w1t", [HT, P, F_DIM], dt.bfloat16, isOutput=False)
    b1_d = nc.declare_dram_parameter("b1t", [P, HT], dt.float32, isOutput=False)
    w2_d = nc.declare_dram_parameter("w2r", [HT, P, CLASSES], dt.bfloat16, isOutput=False)
    b2_d = nc.declare_dram_parameter("b2r", [P, CLASSES], dt.float32, isOutput=False)

    out_d = nc.declare_dram_parameter("out", [BT, P, CLASSES], dt.float32, isOutput=True)
    att_d = nc.declare_dram_parameter("attT", [FT, P, BL], dt.bfloat16, isOutput=True)

    AFT = mybir.ActivationFunctionType
    HB = BL // 2              # 512, matmul N limit

    with tile.TileContext(nc) as tc:
        from contextlib import ExitStack

        with ExitStack() as ctx:
            const_pool = ctx.enter_context(tc.tile_pool(name="const", bufs=1))
            xt_pool = ctx.enter_context(tc.tile_pool(name="xt", bufs=FT))
            wt_pool = ctx.enter_context(tc.tile_pool(name="wt", bufs=FT))
            ht_pool = ctx.enter_context(tc.tile_pool(name="ht", bufs=HT))
            w1_pool = ctx.enter_context(tc.tile_pool(name="w1", bufs=4))
            w2_pool = ctx.enter_context(tc.tile_pool(name="w2", bufs=HT))
            act_pool = ctx.enter_context(tc.tile_pool(name="acts", bufs=2))
            out_pool = ctx.enter_context(tc.tile_pool(name="outs", bufs=3))

            # ---- load xT tiles first (they gate the first matmuls); split
            # the first tiles into half-DMAs and alternate issue between the
            # SP and Activation HWDGE sequencers so transfers ramp at once.
            xt = []
            for f in range(FT):
                xt_t = xt_pool.tile([P, BL], dt.bfloat16, tag="xt", name=f"xt{f}")
                eng = nc.sync if f % 2 == 0 else nc.scalar
                if f < 4:
                    eng.dma_start(xt_t[:, 0:BL // 2], xT_d[f, :, 0:BL // 2])
                    eng2 = nc.scalar if f % 2 == 0 else nc.sync
                    eng2.dma_start(xt_t[:, BL // 2:BL], xT_d[f, :, BL // 2:BL])
                else:
                    eng.dma_start(xt_t[:], xT_d[f, :, :])
                xt.append(xt_t)
                if f == 0:
                    wet_sb = const_pool.tile([P, FT * 96], dt.bfloat16)
                    nc.sync.dma_start(wet_sb[:], wet_d[:, :])

            # ---- remaining constants / small weights ----
            be_sb = const_pool.tile([96, 1], dt.float32)
            nc.sync.dma_start(be_sb[:], be_d[:, :])
            waq_sb = const_pool.tile([96, F_DIM], dt.bfloat16)
            nc.sync.dma_start(waq_sb[:], waq_d[:, :])
            ba0_sb = const_pool.tile([P, FT], dt.float32)
            nc.sync.dma_start(ba0_sb[:], ba0_d[:, :])
            ba1_sb = const_pool.tile([P, FT], dt.float32)
            nc.sync.dma_start(ba1_sb[:], ba1_d[:, :])
            # ---- eT = tanh(We.T @ xT + be), replicated at partitions
            # 0-31 and 64-95 so the two gate matmuls can run as concurrent
            # 32-row PE tiles (T0 / T8) ----
            e_sb = const_pool.tile([96, BL], dt.bfloat16)
            with tc.tile_pool(name="psE", bufs=1, space="PSUM") as psE:
                pe = psE.tile([96, BL], dt.float32, tag="pe", name="pe")
                for f in range(FT):
                    lhs = wet_sb[:, f * 96:(f + 1) * 96]
                    nc.tensor.matmul(pe[:, 0:HB], lhs, xt[f][:, 0:HB],
                                     start=(f == 0), stop=(f == FT - 1))
                    nc.tensor.matmul(pe[:, HB:BL], lhs, xt[f][:, HB:BL],
                                     start=(f == 0), stop=(f == FT - 1))
                nc.scalar.activation(e_sb[:], pe[:], AFT.Tanh, bias=be_sb[:, 0:1])

            # late-needed constants + W1 tiles for the interleaved H-group
            # (emitted after the x/eT block so they don't delay x tiles)
            b1_sb = const_pool.tile([P, HT], dt.float32)
            nc.sync.dma_start(b1_sb[:], b1_d[:, :])
            b2_sb = const_pool.tile([P, CLASSES], dt.float32)
            nc.sync.dma_start(b2_sb[:], b2_d[:, :])
            w1_g0 = []
            for h in range(NG0):
                w1_sb = w1_pool.tile([P, F_DIM], dt.bfloat16, tag="w1", name=f"w1g0_{h}")
                nc.sync.dma_start(w1_sb[:], w1_d[h, :, :])
                w1_g0.append(w1_sb)

            # ---- gate loop (Scalar-chain paced) + interleaved B-matmuls ----
            wt_tiles = [None] * FT
            ht_g0 = []
            with (
                tc.tile_pool(name="psZ", bufs=1, space="PSUM") as psZ,
                tc.tile_pool(name="psB0", bufs=1, space="PSUM") as psB0,
            ):
                phg = []
                for h in range(NG0):
                    phg.append(psB0.tile([P, BL], dt.float32, tag=f"phg{h}",
                                         name=f"phg{h}"))

                for fi in range(FT + LAG):
                    if fi < FT:
                        f = fi
                        zp0 = psZ.tile([P, BL], dt.float32, tag="zp0", name=f"zp0_{f}")
                        zp1 = psZ.tile([P, BL], dt.float32, tag="zp1", name=f"zp1_{f}")
                        l0 = waq_sb[0:E_NODE, f * P:(f + 1) * P]
                        l1 = waq_sb[64:64 + E_NODE, f * P:(f + 1) * P]
                        nc.tensor.matmul(zp0[:, 0:HB], l0, e_sb[0:E_NODE, 0:HB],
                                         start=True, stop=True, tile_position=(0, 0))
                        nc.tensor.matmul(zp1[:, 0:HB], l1, e_sb[64:64 + E_NODE, 0:HB],
                                         start=True, stop=True, tile_position=(64, 0))
                        nc.tensor.matmul(zp0[:, HB:BL], l0, e_sb[0:E_NODE, HB:BL],
                                         start=True, stop=True, tile_position=(0, 0))
                        nc.tensor.matmul(zp1[:, HB:BL], l1, e_sb[64:64 + E_NODE, HB:BL],
                                         start=True, stop=True, tile_position=(64, 0))
                        t0 = act_pool.tile([P, BL], dt.float16, tag="t0", name=f"t0_{f}")
                        t1 = act_pool.tile([P, BL], dt.float16, tag="t1", name=f"t1_{f}")
                        nc.scalar.activation(t0[:], zp0[:], AFT.Tanh, bias=ba0_sb[:, f:f + 1])
                        nc.scalar.activation(t1[:], zp1[:], AFT.Tanh, bias=ba1_sb[:, f:f + 1])
                        if f % 2 == 0:
                            dp = act_pool.tile([P, 2 * BL], dt.float16, tag="dp",
                                               name=f"dp_{f}")
                            d_sl = dp[:, 0:BL]
                        else:
                            d_sl = dp[:, BL:2 * BL]
                        nc.vector.tensor_sub(d_sl[:], t1[:], t0[:])
                        if f % 2 == 1:
                            attp = act_pool.tile([P, 2 * BL], dt.bfloat16, tag="attp",
                                                 bufs=2, name=f"attp_{f}")
                            nc.scalar.activation(attp[:], dp[:], AFT.Sigmoid)
                            for g, fg in ((0, f - 1), (1, f)):
                                att_sl = attp[:, g * BL:(g + 1) * BL]
                                wt_t = wt_pool.tile([P, BL], dt.bfloat16, tag="wt",
                                                    name=f"wt{fg}")
                                nc.vector.tensor_mul(wt_t[:], xt[fg][:], att_sl[:])
                                nc.sync.dma_start(att_d[fg, :, :], att_sl[:])
                                wt_tiles[fg] = wt_t

                    if fi >= LAG:
                        f2 = fi - LAG
                        for h in range(NG0):
                            lhs = w1_g0[h][:, f2 * P:(f2 + 1) * P]
                            nc.tensor.matmul(phg[h][:, 0:HB], lhs,
                                             wt_tiles[f2][:, 0:HB],
                                             start=(f2 == 0), stop=(f2 == FT - 1))
                            nc.tensor.matmul(phg[h][:, HB:BL], lhs,
                                             wt_tiles[f2][:, HB:BL],
                                             start=(f2 == 0), stop=(f2 == FT - 1))

                for h in range(NG0):
                    ht_t = ht_pool.tile([P, BL], dt.bfloat16, tag="ht", name=f"htg{h}")
                    nc.scalar.activation(ht_t[:, 0:HB], phg[h][:, 0:HB], AFT.Relu,
                                         bias=b1_sb[:, h:h + 1])
                    nc.scalar.activation(ht_t[:, HB:BL], phg[h][:, HB:BL], AFT.Relu,
                                         bias=b1_sb[:, h:h + 1])
                    ht_g0.append(ht_t)

            # ---- prefetch W2 ----
            w2_sb = []
            for h in range(HT):
                w2_t = w2_pool.tile([P, CLASSES], dt.bfloat16, tag="w2", name=f"w2_{h}")
                nc.sync.dma_start(w2_t[:], w2_d[h, :, :])
                w2_sb.append(w2_t)

            # ---- phase B remainder: hT = relu(W1.T @ wT + b1) for h >= NG0 ----
            ht_tiles = list(ht_g0)
            with tc.tile_pool(name="psB", bufs=2, space="PSUM") as psB:
                for h in range(NG0, HT):
                    w1_sb = w1_pool.tile([P, F_DIM], dt.bfloat16, tag="w1",
                                         name=f"w1_{h}")
                    nc.sync.dma_start(w1_sb[:], w1_d[h, :, :])
                    ph0 = psB.tile([P, HB], dt.float32, tag="ph0", name=f"ph0_{h}")
                    ph1 = psB.tile([P, HB], dt.float32, tag="ph1", name=f"ph1_{h}")
                    for f in range(FT):
                        lhs = w1_sb[:, f * P:(f + 1) * P]
                        nc.tensor.matmul(ph0[:], lhs, wt_tiles[f][:, 0:HB],
                                         start=(f == 0), stop=(f == FT - 1))
                        nc.tensor.matmul(ph1[:], lhs, wt_tiles[f][:, HB:BL],
                                         start=(f == 0), stop=(f == FT - 1))
                    ht_t = ht_pool.tile([P, BL], dt.bfloat16, tag="ht", name=f"ht{h}")
                    nc.scalar.activation(ht_t[:, 0:HB], ph0[:], AFT.Relu,
                                         bias=b1_sb[:, h:h + 1])
                    nc.scalar.activation(ht_t[:, HB:BL], ph1[:], AFT.Relu,
                                         bias=b1_sb[:, h:h + 1])
                    ht_tiles.append(ht_t)

            # ---- phase C: out = hT.T @ W2 + b2 ----
            with tc.tile_pool(name="psC", bufs=2, space="PSUM") as psC:
                for b in range(BT):
                    po0 = psC.tile([P, CH], dt.float32, tag="o0", name=f"po0_{b}")
                    po1 = psC.tile([P, CH], dt.float32, tag="o1", name=f"po1_{b}")
                    for h in range(HT):
                        lhs = ht_tiles[h][:, b * P:(b + 1) * P]
                        nc.tensor.matmul(po0[:], lhs, w2_sb[h][:, 0:CH],
                                         start=(h == 0), stop=(h == HT - 1))
                        nc.tensor.matmul(po1[:], lhs, w2_sb[h][:, CH:CLASSES],
                                         start=(h == 0), stop=(h == HT - 1))
                    o_sb = out_pool.tile([P, CLASSES], dt.float32, tag="o", name=f"o{b}")
                    nc.vector.tensor_add(o_sb[:, 0:CH], po0[:], b2_sb[:, 0:CH])
                    nc.vector.tensor_add(o_sb[:, CH:CLASSES], po1[:],
                                         b2_sb[:, CH:CLASSES])
                    nc.sync.dma_start(out_d[b, :, :], o_sb[:])

    nc.compile()
    return nc


def _prep_inputs(x, We, be, Wa, ba, W1, b1, W2, b2):
    f32 = np.float32

    xt_all = np.ascontiguousarray(
        x.astype(BF16).reshape(NCORES, BL, FT, P).transpose(0, 2, 3, 1)
    )
    wet3 = np.zeros((P, FT, 96), dtype=np.float32)
    wer = We.reshape(FT, P, E_NODE).transpose(1, 0, 2)   # [P, FT, E]
    wet3[:, :, 0:E_NODE] = wer
    wet3[:, :, 64:64 + E_NODE] = wer
    wet = np.ascontiguousarray(wet3.reshape(P, FT * 96)).astype(BF16)
    be_c = np.zeros((96, 1), dtype=f32)
    be_c[0:E_NODE, 0] = be
    be_c[64:64 + E_NODE, 0] = be
    waq = np.zeros((96, F_DIM), dtype=np.float32)
    waq[0:E_NODE] = Wa[:, :, 0].T
    waq[64:64 + E_NODE] = Wa[:, :, 1].T
    waq = waq.astype(BF16)
    ba0t = np.ascontiguousarray(ba[:, 0].reshape(FT, P).T).astype(f32)
    ba1t = np.ascontiguousarray(ba[:, 1].reshape(FT, P).T).astype(f32)
    w1t = np.ascontiguousarray(
        W1.astype(BF16).reshape(FT, P, HT, P).transpose(2, 1, 0, 3).reshape(HT, P, F_DIM)
    )
    b1t = np.ascontiguousarray(b1.reshape(HT, P).T).astype(f32)
    w2r = np.ascontiguousarray(W2.reshape(HT, P, CLASSES)).astype(BF16)
    b2r = np.ascontiguousarray(np.broadcast_to(b2.astype(f32), (P, CLASSES)))

    shared = {
        "wet": wet, "be": be_c, "waq": waq,
        "ba0t": ba0t, "ba1t": ba1t, "w1t": w1t, "b1t": b1t,
        "w2r": w2r, "b2r": b2r,
    }
    return [dict(shared, xT=np.ascontiguousarray(xt_all[c])) for c in range(NCORES)]


def run_on_hw(inputs, trace=False):
    if "nc" not in _CACHED:
        _CACHED["nc"] = build_nc()
    nc = _CACHED["nc"]
    in_maps = _prep_inputs(**inputs)
    res = run_bass_kernel_spmd(nc, in_maps, core_ids=list(range(NCORES)), trace=trace)

    outs = []
    atts = []
    for c in range(NCORES):
        outs.append(res.results[c]["out"].reshape(BL, CLASSES))
        att_c = res.results[c]["attT"]  # [FT, P, BL] bf16
        atts.append(att_c.transpose(2, 0, 1).reshape(BL, F_DIM))
    output = np.concatenate(outs, axis=0).astype(np.float32)
    attention = np.concatenate(atts, axis=0).astype(np.float32)
    return (output, attention), res


def kernel(**inputs):
    (output, attention), _ = run_on_hw(inputs, trace=False)
    return (output, attention)


# revision 21
# speedup vs baseline: 1.2298x; 1.0194x over previous
"""Trainium2 Bass kernel for the attention-gated MLP (nn_AFS_12549894439384).

Data-parallel over batch across 8 NeuronCores; all parameters replicated.
All matmuls run in bf16 with fp32 PSUM accumulation.

Per-core compute (local batch BL=1024) in transposed [feature, batch] layout:
    eT   = tanh(We.T @ xT + be)                  [32, BL]
    z_a  = Wa[:,:,a].T-tile @ eT  (K=32)         [128, BL] per F-tile
    att  = sigmoid(tanh(z1+ba1) - tanh(z0+ba0))  [128, BL]
    wT   = xT * att                              [128, BL]
    hT   = relu(W1-tile.T @ wT + b1)             [128, BL] per H-tile (K=2048)
    out  = hT-slice.T @ W2 + b2                  [128, 1000] per B-tile (K=2048)

The gate loop is paced by the Scalar engine's tanh/tanh/sigmoid chain
(~3us per F-tile); the first two H-tiles of the W1 matmul are interleaved
into it with a 2-iteration lag so the TensorEngine stays busy during it.
"""

import numpy as np
import ml_dtypes

import concourse.bass as bass
import concourse.mybir as mybir
import concourse.tile as tile
from concourse import bacc
from concourse.bass_utils import run_bass_kernel_spmd

BF16 = ml_dtypes.bfloat16

F_DIM, E_NODE, A_NODE = 2048, 32, 2
HIDDEN, CLASSES, BATCH = 2048, 1000, 8192
NCORES = 8
BL = BATCH // NCORES          # 1024 local batch rows per core
P = 128
FT = F_DIM // P               # 16 feature tiles
HT = HIDDEN // P              # 16 hidden tiles
BT = BL // P                  # 8 local batch tiles
CH = CLASSES // 2             # 500
NG0 = 2                       # H-tiles interleaved into the gate loop
LAG = 2                       # gate-loop iterations between wT and its B-matmuls

_CACHED = {}


def build_nc():
    dt = mybir.dt
    nc = bacc.Bacc("TRN2", target_bir_lowering=False, debug=False)

    xT_d = nc.declare_dram_parameter("xT", [FT, P, BL], dt.bfloat16, isOutput=False)
    wet_d = nc.declare_dram_parameter("wet", [P, FT * 96], dt.bfloat16, isOutput=False)
    be_d = nc.declare_dram_parameter("be", [96, 1], dt.float32, isOutput=False)
    waq_d = nc.declare_dram_parameter("waq", [96, F_DIM], dt.bfloat16, isOutput=False)
    ba0_d = nc.declare_dram_parameter("ba0t", [P, FT], dt.float32, isOutput=False)
    ba1_d = nc.declare_dram_parameter("ba1t", [P, FT], dt.float32, isOutput=False)
    w1_d = nc.declare_dram_parameter("w1t", [HT, P, F_DIM], dt.bfloat16, isOutput=False)
    b1_d = nc.declare_dram_parameter("b1t", [P, HT], dt.float32, isOutput=False)
    w2_d = nc.declare_dram_parameter("w2r", [HT, P, CLASSES], dt.bfloat16, isOutput=False)
    b2_d = nc.declare_dram_parameter("b2r", [P, CLASSES], dt.float32, isOutput=False)

    out_d = nc.declare_dram_parameter("out", [BT, P, CLASSES], dt.float32, isOutput=True)
    att_d = nc.declare_dram_parameter("attT", [FT, P, BL], dt.bfloat16, isOutput=True)

    AFT = mybir.ActivationFunctionType
    HB = BL // 2              # 512, matmul N limit

    with tile.TileContext(nc) as tc:
        from contextlib import ExitStack

        with ExitStack() as ctx:
            const_pool = ctx.enter_context(tc.tile_pool(name="const", bufs=1))
            xt_pool = ctx.enter_context(tc.tile_pool(name="xt", bufs=FT))
            wt_pool = ctx.enter_context(tc.tile_pool(name="wt", bufs=FT))
            ht_pool = ctx.enter_context(tc.tile_pool(name="ht", bufs=HT))
            w1_pool = ctx.enter_context(tc.tile_pool(name="w1", bufs=4))
            w2_pool = ctx.enter_context(tc.tile_pool(name="w2", bufs=HT))
            act_pool = ctx.enter_context(tc.tile_pool(name="acts", bufs=2))
            out_pool = ctx.enter_context(tc.tile_pool(name="outs", bufs=3))

            # ---- load xT tiles first (they gate the first matmuls); split
            # the first tiles into half-DMAs and alternate issue between the
            # SP and Activation HWDGE sequencers so transfers ramp at once.
            xt = []
            for f in range(FT):
                xt_t = xt_pool.tile([P, BL], dt.bfloat16, tag="xt", name=f"xt{f}")
                eng = nc.sync if f % 2 == 0 else nc.scalar
                if f < 4:
                    eng.dma_start(xt_t[:, 0:BL // 2], xT_d[f, :, 0:BL // 2])
                    eng2 = nc.scalar if f % 2 == 0 else nc.sync
                    eng2.dma_start(xt_t[:, BL // 2:BL], xT_d[f, :, BL // 2:BL])
                else:
                    eng.dma_start(xt_t[:], xT_d[f, :, :])
                xt.append(xt_t)
                if f == 0:
                    wet_sb = const_pool.tile([P, FT * 96], dt.bfloat16)
                    nc.sync.dma_start(wet_sb[:], wet_d[:, :])

            # ---- remaining constants / small weights ----
            be_sb = const_pool.tile([96, 1], dt.float32)
            nc.sync.dma_start(be_sb[:], be_d[:, :])
            waq_sb = const_pool.tile([96, F_DIM], dt.bfloat16)
            nc.sync.dma_start(waq_sb[:], waq_d[:, :])
            ba0_sb = const_pool.tile([P, FT], dt.float32)
            nc.sync.dma_start(ba0_sb[:], ba0_d[:, :])
            ba1_sb = const_pool.tile([P, FT], dt.float32)
            nc.sync.dma_start(ba1_sb[:], ba1_d[:, :])
            b1_sb = const_pool.tile([P, HT], dt.float32)
            nc.sync.dma_start(b1_sb[:], b1_d[:, :])
            b2_sb = const_pool.tile([P, CLASSES], dt.float32)
            nc.sync.dma_start(b2_sb[:], b2_d[:, :])

            # W1 tiles for the interleaved H-group, prefetched early
            w1_g0 = []
            for h in range(NG0):
                w1_sb = w1_pool.tile([P, F_DIM], dt.bfloat16, tag="w1", name=f"w1g0_{h}")
                nc.sync.dma_start(w1_sb[:], w1_d[h, :, :])
                w1_g0.append(w1_sb)

            # ---- eT = tanh(We.T @ xT + be), replicated at partitions
            # 0-31 and 64-95 so the two gate matmuls can run as concurrent
            # 32-row PE tiles (T0 / T8) ----
            e_sb = const_pool.tile([96, BL], dt.bfloat16)
            with tc.tile_pool(name="psE", bufs=1, space="PSUM") as psE:
                pe = psE.tile([96, BL], dt.float32, tag="pe", name="pe")
                for f in range(FT):
                    lhs = wet_sb[:, f * 96:(f + 1) * 96]
                    nc.tensor.matmul(pe[:, 0:HB], lhs, xt[f][:, 0:HB],
                                     start=(f == 0), stop=(f == FT - 1))
                    nc.tensor.matmul(pe[:, HB:BL], lhs, xt[f][:, HB:BL],
                                     start=(f == 0), stop=(f == FT - 1))
                nc.scalar.activation(e_sb[:], pe[:], AFT.Tanh, bias=be_sb[:, 0:1])

            # ---- gate loop (Scalar-chain paced) + interleaved B-matmuls ----
            wt_tiles = [None] * FT
            ht_g0 = []
            with (
                tc.tile_pool(name="psZ", bufs=1, space="PSUM") as psZ,
                tc.tile_pool(name="psB0", bufs=1, space="PSUM") as psB0,
            ):
                phg = []
                for h in range(NG0):
                    phg.append(psB0.tile([P, BL], dt.float32, tag=f"phg{h}",
                                         name=f"phg{h}"))

                for fi in range(FT + LAG):
                    if fi < FT:
                        f = fi
                        zp0 = psZ.tile([P, BL], dt.float32, tag="zp0", name=f"zp0_{f}")
                        zp1 = psZ.tile([P, BL], dt.float32, tag="zp1", name=f"zp1_{f}")
                        l0 = waq_sb[0:E_NODE, f * P:(f + 1) * P]
                        l1 = waq_sb[64:64 + E_NODE, f * P:(f + 1) * P]
                        nc.tensor.matmul(zp0[:, 0:HB], l0, e_sb[0:E_NODE, 0:HB],
                                         start=True, stop=True, tile_position=(0, 0))
                        nc.tensor.matmul(zp1[:, 0:HB], l1, e_sb[64:64 + E_NODE, 0:HB],
                                         start=True, stop=True, tile_position=(64, 0))
                        nc.tensor.matmul(zp0[:, HB:BL], l0, e_sb[0:E_NODE, HB:BL],
                                         start=True, stop=True, tile_position=(0, 0))
                        nc.tensor.matmul(zp1[:, HB:BL], l1, e_sb[64:64 + E_NODE, HB:BL],
                                         start=True, stop=True, tile_position=(64, 0))
                        t0 = act_pool.tile([P, BL], dt.float16, tag="t0", name=f"t0_{f}")
                        t1 = act_pool.tile([P, BL], dt.float16, tag="t1", name=f"t1_{f}")
                        nc.scalar.activation(t0[:], zp0[:], AFT.Tanh, bias=ba0_sb[:, f:f + 1])
                        nc.scalar.activation(t1[:], zp1[:], AFT.Tanh, bias=ba1_sb[:, f:f + 1])
                        d_t = act_pool.tile([P, BL], dt.float16, tag="d", name=f"d_{f}")
                        nc.vector.tensor_sub(d_t[:], t1[:], t0[:])
                        att_sb = act_pool.tile([P, BL], dt.bfloat16, tag="att", bufs=3,
                                               name=f"att_{f}")
                        nc.scalar.activation(att_sb[:], d_t[:], AFT.Sigmoid)
                        wt_t = wt_pool.tile([P, BL], dt.bfloat16, tag="wt", name=f"wt{f}")
                        nc.vector.tensor_mul(wt_t[:], xt[f][:], att_sb[:])
                        nc.sync.dma_start(att_d[f, :, :], att_sb[:])
                        wt_tiles[f] = wt_t

                    if fi >= LAG:
                        f2 = fi - LAG
                        for h in range(NG0):
                            lhs = w1_g0[h][:, f2 * P:(f2 + 1) * P]
                            nc.tensor.matmul(phg[h][:, 0:HB], lhs,
                                             wt_tiles[f2][:, 0:HB],
                                             start=(f2 == 0), stop=(f2 == FT - 1))
                            nc.tensor.matmul(phg[h][:, HB:BL], lhs,
                                             wt_tiles[f2][:, HB:BL],
                                             start=(f2 == 0), stop=(f2 == FT - 1))

                for h in range(NG0):
                    ht_t = ht_pool.tile([P, BL], dt.bfloat16, tag="ht", name=f"htg{h}")
                    nc.scalar.activation(ht_t[:, 0:HB], phg[h][:, 0:HB], AFT.Relu,
                                         bias=b1_sb[:, h:h + 1])
                    nc.scalar.activation(ht_t[:, HB:BL], phg[h][:, HB:BL], AFT.Relu,
                                         bias=b1_sb[:, h:h + 1])
                    ht_g0.append(ht_t)

            # ---- prefetch W2 ----
            w2_sb = []
            for h in range(HT):
                w2_t = w2_pool.tile([P, CLASSES], dt.bfloat16, tag="w2", name=f"w2_{h}")
                nc.sync.dma_start(w2_t[:], w2_d[h, :, :])
                w2_sb.append(w2_t)

            # ---- phase B remainder: hT = relu(W1.T @ wT + b1) for h >= NG0 ----
            ht_tiles = list(ht_g0)
            with tc.tile_pool(name="psB", bufs=2, space="PSUM") as psB:
                for h in range(NG0, HT):
                    w1_sb = w1_pool.tile([P, F_DIM], dt.bfloat16, tag="w1",
                                         name=f"w1_{h}")
                    nc.sync.dma_start(w1_sb[:], w1_d[h, :, :])
                    ph0 = psB.tile([P, HB], dt.float32, tag="ph0", name=f"ph0_{h}")
                    ph1 = psB.tile([P, HB], dt.float32, tag="ph1", name=f"ph1_{h}")
                    for f in range(FT):
                        lhs = w1_sb[:, f * P:(f + 1) * P]
                        nc.tensor.matmul(ph0[:], lhs, wt_tiles[f][:, 0:HB],
                                         start=(f == 0), stop=(f == FT - 1))
                        nc.tensor.matmul(ph1[:], lhs, wt_tiles[f][:, HB:BL],
                                         start=(f == 0), stop=(f == FT - 1))
                    ht_t = ht_pool.tile([P, BL], dt.bfloat16, tag="ht", name=f"ht{h}")
                    nc.scalar.activation(ht_t[:, 0:HB], ph0[:], AFT.Relu,
                                         bias=b1_sb[:, h:h + 1])
                    nc.scalar.activation(ht_t[:, HB:BL], ph1[:], AFT.Relu,
                                         bias=b1_sb[:, h:h + 1])
                    ht_tiles.append(ht_t)

            # ---- phase C: out = hT.T @ W2 + b2 ----
            with tc.tile_pool(name="psC", bufs=2, space="PSUM") as psC:
                for b in range(BT):
                    po0 = psC.tile([P, CH], dt.float32, tag="o0", name=f"po0_{b}")
                    po1 = psC.tile([P, CH], dt.float32, tag="o1", name=f"po1_{b}")
                    for h in range(HT):
                        lhs = ht_tiles[h][:, b * P:(b + 1) * P]
                        nc.tensor.matmul(po0[:], lhs, w2_sb[h][:, 0:CH],
                                         start=(h == 0), stop=(h == HT - 1))
                        nc.tensor.matmul(po1[:], lhs, w2_sb[h][:, CH:CLASSES],
                                         start=(h == 0), stop=(h == HT - 1))
                    o_sb = out_pool.tile([P, CLASSES], dt.float32, tag="o", name=f"o{b}")
                    nc.vector.tensor_add(o_sb[:, 0:CH], po0[:], b2_sb[:, 0:CH])
                    nc.vector.tensor_add(o_sb[:, CH:CLASSES], po1[:],
                                         b2_sb[:, CH:CLASSES])
                    nc.sync.dma_start(out_d[b, :, :], o_sb[:])

    nc.compile()
    return nc


def _prep_inputs(x, We, be, Wa, ba, W1, b1, W2, b2):
    f32 = np.float32

    xt_all = np.ascontiguousarray(
        x.astype(BF16).reshape(NCORES, BL, FT, P).transpose(0, 2, 3, 1)
    )
    wet3 = np.zeros((P, FT, 96), dtype=np.float32)
    wer = We.reshape(FT, P, E_NODE).transpose(1, 0, 2)   # [P, FT, E]
    wet3[:, :, 0:E_NODE] = wer
    wet3[:, :, 64:64 + E_NODE] = wer
    wet = np.ascontiguousarray(wet3.reshape(P, FT * 96)).astype(BF16)
    be_c = np.zeros((96, 1), dtype=f32)
    be_c[0:E_NODE, 0] = be
    be_c[64:64 + E_NODE, 0] = be
    waq = np.zeros((96, F_DIM), dtype=np.float32)
    waq[0:E_NODE] = Wa[:, :, 0].T
    waq[64:64 + E_NODE] = Wa[:, :, 1].T
    waq = waq.astype(BF16)
    ba0t = np.ascontiguousarray(ba[:, 0].reshape(FT, P).T).astype(f32)
    ba1t = np.ascontiguousarray(ba[:, 1].reshape(FT, P).T).astype(f32)
    w1t = np.ascontiguousarray(
        W1.astype(BF16).reshape(FT, P, HT, P).transpose(2, 1, 0, 3).reshape(HT, P, F_DIM)
    )
    b1t = np.ascontiguousarray(b1.reshape(HT, P).T).astype(f32)
    w2r = np.ascontiguousarray(W2.reshape(HT, P, CLASSES)).astype(BF16)
    b2r = np.ascontiguousarray(np.broadcast_to(b2.astype(f32), (P, CLASSES)))

    shared = {
        "wet": wet, "be": be_c, "waq": waq,
        "ba0t": ba0t, "ba1t": ba1t, "w1t": w1t, "b1t": b1t,
        "w2r": w2r, "b2r": b2r,
    }
    return [dict(shared, xT=np.ascontiguousarray(xt_all[c])) for c in range(NCORES)]


def run_on_hw(inputs, trace=False):
    if "nc" not in _CACHED:
        _CACHED["nc"] = build_nc()
    nc = _CACHED["nc"]
    inputs = {k: np.asarray(v, dtype=np.float32) for k, v in inputs.items()}
    in_maps = _prep_inputs(**inputs)
    res = run_bass_kernel_spmd(nc, in_maps, core_ids=list(range(NCORES)), trace=trace)

    outs = []
    atts = []
    for c in range(NCORES):
        outs.append(res.results[c]["out"].reshape(BL, CLASSES))
        att_c = res.results[c]["attT"]  # [FT, P, BL] bf16
        atts.append(att_c.transpose(2, 0, 1).reshape(BL, F_DIM))
    output = np.concatenate(outs, axis=0).astype(np.float32)
    attention = np.concatenate(atts, axis=0).astype(np.float32)
    return (output, attention), res


def kernel(**inputs):
    (output, attention), _ = run_on_hw(inputs, trace=False)
    return (output, attention)


# revision 22
# speedup vs baseline: 1.2368x; 1.0057x over previous
"""Trainium2 Bass kernel for the attention-gated MLP (nn_AFS_12549894439384).

Data-parallel over batch across 8 NeuronCores; all parameters replicated.
All matmuls run in bf16 with fp32 PSUM accumulation.

Per-core compute (local batch BL=1024) in transposed [feature, batch] layout:
    eT   = tanh(We.T @ xT + be)                  [32, BL]
    z_a  = Wa[:,:,a].T-tile @ eT  (K=32)         [128, BL] per F-tile
    att  = sigmoid(tanh(z1+ba1) - tanh(z0+ba0))  [128, BL]
    wT   = xT * att                              [128, BL]
    hT   = relu(W1-tile.T @ wT + b1)             [128, BL] per H-tile (K=2048)
    out  = hT-slice.T @ W2 + b2                  [128, 1000] per B-tile (K=2048)

The gate loop is paced by the Scalar engine's tanh/tanh/sigmoid chain
(~3us per F-tile); the first two H-tiles of the W1 matmul are interleaved
into it with a 2-iteration lag so the TensorEngine stays busy during it.
"""

import numpy as np
import ml_dtypes

import concourse.bass as bass
import concourse.mybir as mybir
import concourse.tile as tile
from concourse import bacc
from concourse.bass_utils import run_bass_kernel_spmd

BF16 = ml_dtypes.bfloat16

F_DIM, E_NODE, A_NODE = 2048, 32, 2
HIDDEN, CLASSES, BATCH = 2048, 1000, 8192
NCORES = 8
BL = BATCH // NCORES          # 1024 local batch rows per core
P = 128
FT = F_DIM // P               # 16 feature tiles
HT = HIDDEN // P              # 16 hidden tiles
BT = BL // P                  # 8 local batch tiles
CH = CLASSES // 2             # 500
NG0 = 2                       # H-tiles interleaved into the gate loop
LAG = 3                       # gate-loop iterations between wT and its B-matmuls

_CACHED = {}


def build_nc():
    dt = mybir.dt
    nc = bacc.Bacc("TRN2", target_bir_lowering=False, debug=False)

    xT_d = nc.declare_dram_parameter("xT", [FT, P, BL], dt.bfloat16, isOutput=False)
    wet_d = nc.declare_dram_parameter("wet", [P, FT * 96], dt.bfloat16, isOutput=False)
    be_d = nc.declare_dram_parameter("be", [96, 1], dt.float32, isOutput=False)
    waq_d = nc.declare_dram_parameter("waq", [96, F_DIM], dt.bfloat16, isOutput=False)
    ba0_d = nc.declare_dram_parameter("ba0t", [P, FT], dt.float32, isOutput=False)
    ba1_d = nc.declare_dram_parameter("ba1t", [P, FT], dt.float32, isOutput=False)
    w1_d = nc.declare_dram_parameter("w1t", [HT, P, F_DIM], dt.bfloat16, isOutput=False)
    b1_d = nc.declare_dram_parameter("b1t", [P, HT], dt.float32, isOutput=False)
    w2_d = nc.declare_dram_parameter("w2r", [HT, P, CLASSES], dt.bfloat16, isOutput=False)
    b2_d = nc.declare_dram_parameter("b2r", [P, CLASSES], dt.float32, isOutput=False)

    out_d = nc.declare_dram_parameter("out", [BT, P, CLASSES], dt.float32, isOutput=True)
    att_d = nc.declare_dram_parameter("attT", [FT, P, BL], dt.bfloat16, isOutput=True)

    AFT = mybir.ActivationFunctionType
    HB = BL // 2              # 512, matmul N limit

    with tile.TileContext(nc) as tc:
        from contextlib import ExitStack

        with ExitStack() as ctx:
            const_pool = ctx.enter_context(tc.tile_pool(name="const", bufs=1))
            xt_pool = ctx.enter_context(tc.tile_pool(name="xt", bufs=FT))
            wt_pool = ctx.enter_context(tc.tile_pool(name="wt", bufs=FT))
            ht_pool = ctx.enter_context(tc.tile_pool(name="ht", bufs=HT))
            w1_pool = ctx.enter_context(tc.tile_pool(name="w1", bufs=4))
            w2_pool = ctx.enter_context(tc.tile_pool(name="w2", bufs=HT))
            act_pool = ctx.enter_context(tc.tile_pool(name="acts", bufs=2))
            out_pool = ctx.enter_context(tc.tile_pool(name="outs", bufs=3))

            # ---- load xT tiles first (they gate the first matmuls); split
            # the first tiles into half-DMAs and alternate issue between the
            # SP and Activation HWDGE sequencers so transfers ramp at once.
            xt = []
            for f in range(FT):
                xt_t = xt_pool.tile([P, BL], dt.bfloat16, tag="xt", name=f"xt{f}")
                eng = nc.sync if f % 2 == 0 else nc.scalar
                if f < 4:
                    eng.dma_start(xt_t[:, 0:BL // 2], xT_d[f, :, 0:BL // 2])
                    eng2 = nc.scalar if f % 2 == 0 else nc.sync
                    eng2.dma_start(xt_t[:, BL // 2:BL], xT_d[f, :, BL // 2:BL])
                else:
                    eng.dma_start(xt_t[:], xT_d[f, :, :])
                xt.append(xt_t)
                if f == 0:
                    wet_sb = const_pool.tile([P, FT * 96], dt.bfloat16)
                    nc.sync.dma_start(wet_sb[:], wet_d[:, :])

            # ---- remaining constants / small weights ----
            be_sb = const_pool.tile([96, 1], dt.float32)
            nc.sync.dma_start(be_sb[:], be_d[:, :])
            waq_sb = const_pool.tile([96, F_DIM], dt.bfloat16)
            nc.sync.dma_start(waq_sb[:], waq_d[:, :])
            ba0_sb = const_pool.tile([P, FT], dt.float32)
            nc.sync.dma_start(ba0_sb[:], ba0_d[:, :])
            ba1_sb = const_pool.tile([P, FT], dt.float32)
            nc.sync.dma_start(ba1_sb[:], ba1_d[:, :])
            b1_sb = const_pool.tile([P, HT], dt.float32)
            nc.sync.dma_start(b1_sb[:], b1_d[:, :])
            b2_sb = const_pool.tile([P, CLASSES], dt.float32)
            nc.sync.dma_start(b2_sb[:], b2_d[:, :])

            # W1 tiles for the interleaved H-group, prefetched early
            w1_g0 = []
            for h in range(NG0):
                w1_sb = w1_pool.tile([P, F_DIM], dt.bfloat16, tag="w1", name=f"w1g0_{h}")
                nc.sync.dma_start(w1_sb[:], w1_d[h, :, :])
                w1_g0.append(w1_sb)

            # ---- eT = tanh(We.T @ xT + be), replicated at partitions
            # 0-31 and 64-95 so the two gate matmuls can run as concurrent
            # 32-row PE tiles (T0 / T8) ----
            e_sb = const_pool.tile([96, BL], dt.bfloat16)
            with tc.tile_pool(name="psE", bufs=1, space="PSUM") as psE:
                pe = psE.tile([96, BL], dt.float32, tag="pe", name="pe")
                for f in range(FT):
                    lhs = wet_sb[:, f * 96:(f + 1) * 96]
                    nc.tensor.matmul(pe[:, 0:HB], lhs, xt[f][:, 0:HB],
                                     start=(f == 0), stop=(f == FT - 1))
                    nc.tensor.matmul(pe[:, HB:BL], lhs, xt[f][:, HB:BL],
                                     start=(f == 0), stop=(f == FT - 1))
                nc.scalar.activation(e_sb[:], pe[:], AFT.Tanh, bias=be_sb[:, 0:1])

            # ---- gate loop (Scalar-chain paced) + interleaved B-matmuls ----
            wt_tiles = [None] * FT
            ht_g0 = []
            with (
                tc.tile_pool(name="psZ", bufs=1, space="PSUM") as psZ,
                tc.tile_pool(name="psB0", bufs=1, space="PSUM") as psB0,
            ):
                phg = []
                for h in range(NG0):
                    phg.append(psB0.tile([P, BL], dt.float32, tag=f"phg{h}",
                                         name=f"phg{h}"))

                for fi in range(FT + LAG):
                    if fi < FT:
                        f = fi
                        zp0 = psZ.tile([P, BL], dt.float32, tag="zp0", name=f"zp0_{f}")
                        zp1 = psZ.tile([P, BL], dt.float32, tag="zp1", name=f"zp1_{f}")
                        l0 = waq_sb[0:E_NODE, f * P:(f + 1) * P]
                        l1 = waq_sb[64:64 + E_NODE, f * P:(f + 1) * P]
                        nc.tensor.matmul(zp0[:, 0:HB], l0, e_sb[0:E_NODE, 0:HB],
                                         start=True, stop=True, tile_position=(0, 0))
                        nc.tensor.matmul(zp1[:, 0:HB], l1, e_sb[64:64 + E_NODE, 0:HB],
                                         start=True, stop=True, tile_position=(64, 0))
                        nc.tensor.matmul(zp0[:, HB:BL], l0, e_sb[0:E_NODE, HB:BL],
                                         start=True, stop=True, tile_position=(0, 0))
                        nc.tensor.matmul(zp1[:, HB:BL], l1, e_sb[64:64 + E_NODE, HB:BL],
                                         start=True, stop=True, tile_position=(64, 0))
                        t0 = act_pool.tile([P, BL], dt.float16, tag="t0", name=f"t0_{f}")
                        t1 = act_pool.tile([P, BL], dt.float16, tag="t1", name=f"t1_{f}")
                        nc.scalar.activation(t0[:], zp0[:], AFT.Tanh, bias=ba0_sb[:, f:f + 1])
                        nc.scalar.activation(t1[:], zp1[:], AFT.Tanh, bias=ba1_sb[:, f:f + 1])
                        d_t = act_pool.tile([P, BL], dt.float16, tag="d", name=f"d_{f}")
                        nc.vector.tensor_sub(d_t[:], t1[:], t0[:])
                        att_sb = act_pool.tile([P, BL], dt.bfloat16, tag="att", bufs=3,
                                               name=f"att_{f}")
                        nc.scalar.activation(att_sb[:], d_t[:], AFT.Sigmoid)
                        wt_t = wt_pool.tile([P, BL], dt.bfloat16, tag="wt", name=f"wt{f}")
                        nc.vector.tensor_mul(wt_t[:], xt[f][:], att_sb[:])
                        nc.sync.dma_start(att_d[f, :, :], att_sb[:])
                        wt_tiles[f] = wt_t

                    if fi >= LAG:
                        f2 = fi - LAG
                        for h in range(NG0):
                            lhs = w1_g0[h][:, f2 * P:(f2 + 1) * P]
                            nc.tensor.matmul(phg[h][:, 0:HB], lhs,
                                             wt_tiles[f2][:, 0:HB],
                                             start=(f2 == 0), stop=(f2 == FT - 1))
                            nc.tensor.matmul(phg[h][:, HB:BL], lhs,
                                             wt_tiles[f2][:, HB:BL],
                                             start=(f2 == 0), stop=(f2 == FT - 1))

                for h in range(NG0):
                    ht_t = ht_pool.tile([P, BL], dt.bfloat16, tag="ht", name=f"htg{h}")
                    nc.scalar.activation(ht_t[:, 0:HB], phg[h][:, 0:HB], AFT.Relu,
                                         bias=b1_sb[:, h:h + 1])
                    nc.scalar.activation(ht_t[:, HB:BL], phg[h][:, HB:BL], AFT.Relu,
                                         bias=b1_sb[:, h:h + 1])
                    ht_g0.append(ht_t)

            # ---- prefetch W2 ----
            w2_sb = []
            for h in range(HT):
                w2_t = w2_pool.tile([P, CLASSES], dt.bfloat16, tag="w2", name=f"w2_{h}")
                nc.sync.dma_start(w2_t[:], w2_d[h, :, :])
                w2_sb.append(w2_t)

            # ---- phase B remainder: hT = relu(W1.T @ wT + b1) for h >= NG0 ----
            ht_tiles = list(ht_g0)
            with tc.tile_pool(name="psB", bufs=2, space="PSUM") as psB:
                for h in range(NG0, HT):
                    w1_sb = w1_pool.tile([P, F_DIM], dt.bfloat16, tag="w1",
                                         name=f"w1_{h}")
                    nc.sync.dma_start(w1_sb[:], w1_d[h, :, :])
                    ph0 = psB.tile([P, HB], dt.float32, tag="ph0", name=f"ph0_{h}")
                    ph1 = psB.tile([P, HB], dt.float32, tag="ph1", name=f"ph1_{h}")
                    for f in range(FT):
                        lhs = w1_sb[:, f * P:(f + 1) * P]
                        nc.tensor.matmul(ph0[:], lhs, wt_tiles[f][:, 0:HB],
                                         start=(f == 0), stop=(f == FT - 1))
                        nc.tensor.matmul(ph1[:], lhs, wt_tiles[f][:, HB:BL],
                                         start=(f == 0), stop=(f == FT - 1))
                    ht_t = ht_pool.tile([P, BL], dt.bfloat16, tag="ht", name=f"ht{h}")
                    nc.scalar.activation(ht_t[:, 0:HB], ph0[:], AFT.Relu,
                                         bias=b1_sb[:, h:h + 1])
                    nc.scalar.activation(ht_t[:, HB:BL], ph1[:], AFT.Relu,
                                         bias=b1_sb[:, h:h + 1])
                    ht_tiles.append(ht_t)

            # ---- phase C: out = hT.T @ W2 + b2 ----
            with tc.tile_pool(name="psC", bufs=2, space="PSUM") as psC:
                for b in range(BT):
                    po0 = psC.tile([P, CH], dt.float32, tag="o0", name=f"po0_{b}")
                    po1 = psC.tile([P, CH], dt.float32, tag="o1", name=f"po1_{b}")
                    for h in range(HT):
                        lhs = ht_tiles[h][:, b * P:(b + 1) * P]
                        nc.tensor.matmul(po0[:], lhs, w2_sb[h][:, 0:CH],
                                         start=(h == 0), stop=(h == HT - 1))
                        nc.tensor.matmul(po1[:], lhs, w2_sb[h][:, CH:CLASSES],
                                         start=(h == 0), stop=(h == HT - 1))
                    o_sb = out_pool.tile([P, CLASSES], dt.float32, tag="o", name=f"o{b}")
                    nc.vector.tensor_add(o_sb[:, 0:CH], po0[:], b2_sb[:, 0:CH])
                    nc.vector.tensor_add(o_sb[:, CH:CLASSES], po1[:],
                                         b2_sb[:, CH:CLASSES])
                    nc.sync.dma_start(out_d[b, :, :], o_sb[:])

    nc.compile()
    return nc


def _prep_inputs(x, We, be, Wa, ba, W1, b1, W2, b2):
    f32 = np.float32

    xt_all = np.ascontiguousarray(
        x.astype(BF16).reshape(NCORES, BL, FT, P).transpose(0, 2, 3, 1)
    )
    wet3 = np.zeros((P, FT, 96), dtype=np.float32)
    wer = We.reshape(FT, P, E_NODE).transpose(1, 0, 2)   # [P, FT, E]
    wet3[:, :, 0:E_NODE] = wer
    wet3[:, :, 64:64 + E_NODE] = wer
    wet = np.ascontiguousarray(wet3.reshape(P, FT * 96)).astype(BF16)
    be_c = np.zeros((96, 1), dtype=f32)
    be_c[0:E_NODE, 0] = be
    be_c[64:64 + E_NODE, 0] = be
    waq = np.zeros((96, F_DIM), dtype=np.float32)
    waq[0:E_NODE] = Wa[:, :, 0].T
    waq[64:64 + E_NODE] = Wa[:, :, 1].T
    waq = waq.astype(BF16)
    ba0t = np.ascontiguousarray(ba[:, 0].reshape(FT, P).T).astype(f32)
    ba1t = np.ascontiguousarray(ba[:, 1].reshape(FT, P).T).astype(f32)
    w1t = np.ascontiguousarray(
        W1.astype(BF16).reshape(FT, P, HT, P).transpose(2, 1, 0, 3).reshape(HT, P, F_DIM)
    )
    b1t = np.ascontiguousarray(b1.reshape(HT, P).T).astype(f32)
    w2r = np.ascontiguousarray(W2.reshape(HT, P, CLASSES)).astype(BF16)
    b2r = np.ascontiguousarray(np.broadcast_to(b2.astype(f32), (P, CLASSES)))

    shared = {
        "wet": wet, "be": be_c, "waq": waq,
        "ba0t": ba0t, "ba1t": ba1t, "w1t": w1t, "b1t": b1t,
        "w2r": w2r, "b2r": b2r,
    }
    return [dict(shared, xT=np.ascontiguousarray(xt_all[c])) for c in range(NCORES)]


def run_on_hw(inputs, trace=False):
    if "nc" not in _CACHED:
        _CACHED["nc"] = build_nc()
    nc = _CACHED["nc"]
    inputs = {k: np.asarray(v, dtype=np.float32) for k, v in inputs.items()}
    in_maps = _prep_inputs(**inputs)
    res = run_bass_kernel_spmd(nc, in_maps, core_ids=list(range(NCORES)), trace=trace)

    outs = []
    atts = []
    for c in range(NCORES):
        outs.append(res.results[c]["out"].reshape(BL, CLASSES))
        att_c = res.results[c]["attT"]  # [FT, P, BL] bf16
        atts.append(att_c.transpose(2, 0, 1).reshape(BL, F_DIM))
    output = np.concatenate(outs, axis=0).astype(np.float32)
    attention = np.concatenate(atts, axis=0).astype(np.float32)
    return (output, attention), res


def kernel(**inputs):
    (output, attention), _ = run_on_hw(inputs, trace=False)
    return (output, attention)
